# revision 4
# baseline (speedup 1.0000x reference)
"""Cumulative VWAP kernel for Trainium2 (Bass/Tile), data-parallel over 8 cores.

vwap[:, t] = cumsum(s*v)[:, t] / (cumsum(v)[:, t] + 1e-8),  vwap[:, 0] = s[:, 0]

Sharding: num_paths (axis 0) split evenly across 8 NeuronCores; the cumsum
runs along the time axis, which stays local to each core (no collectives).

v2: u8-coded I/O.  The problem is HBM-bound; the rel-err budget (2e-2) is
spent on 8-bit storage codes instead of fp16:
  * stock  -> x8 = round((s-30)*255/30)   (affine code; s in [30,60))
  * volume -> k8 = round(v*255/8e6)       (pure-scale code; v in [1e6,8e6))
  * output -> o8 with vwap = o8*(30/255)+30
The affine offsets cancel in the VWAP algebra: with s-30 = (30/255)*x and
v = (8e6/255)*k,  (vwap-30)*255/30 = cumsum(x*k)/cumsum(k), so the device
works on raw codes only.  Measured end-to-end rel err ~3.8e-3 (the per-
element quantization averages out in the sums) -- 3x better than the fp16
baseline while moving half the bytes.

Loads/stores are SWDGE (gpsimd) cast-DMAs: u8 in HBM <-> fp16 in SBUF,
round-to-nearest on the store side (verified on HW).  1.5 MiB/tile HBM-side
vs 3 MiB for the fp16 baseline.

The two cumsums run as hand-written custom-DVE uop programs in the 2X_1PORT
perf mode (2 fp16 code pairs/cycle), with a power-of-2 scale (CONST_0 <-
s0 immediate) folded into the scan so the running sums fit fp16:
  PV2XS: pv = cumsum(x*k)*2^-12   (fp16, max |pv| < 2^16 ok)
  V2XS : vc = cumsum(k)*2^-5      (fp16)
The 2^17 net scale plus the output code scale folds into the Exp bias:
  r = exp(-ln(vc) + 7*ln2) = 2^7/vc;  oc = pv*r = (vwap-30)*255/30.
oc[:,0] = x_0*(1 +- 6e-4) rounds to exactly x_0 in the u8 store, so no
explicit t==0 column fix is needed.

Per-core dataflow, per [128, 4096] tile (16 tiles per core):
  DMA(gpsimd): x8 -> xt fp16, k8 -> kt fp16 (cast loads, issued 2 tiles
               ahead of the store in the Pool queue to dodge head-of-line
               blocking on the store's data dependency)
  DVE : pv = PV2XS(xt, kt)        fp16, ~2.2 us
  DVE : vc = V2XS(kt, xt)         fp16 (xt drained; keeps TwoSrc class)
  ACT : ln = Ln(vc)               f32
  ACT : r  = Exp(-ln + 7ln2)      fp16 (in-place into vc's buffer)
  DVE : oc = pv * r               fp16 tensor_tensor 2x
  DMA(gpsimd): oc -> o8 (cast store, RN + [0,255] saturation)
"""

import numpy as np

NUM_PATHS = 16384
TIME = 4096
N_CORES = 8
ROWS = NUM_PATHS // N_CORES  # rows per core
P = 128  # SBUF partitions

C_PV = float(2.0 ** -12)
C_V = float(2.0 ** -5)
C_LN = float(2.0 ** -7)

_CACHE = {}

_COMBINED_SET = "natural_log_exp_and_others"


def _single_act_set_bacc():
    import concourse.bacc as bacc

    class SingleActSetBacc(bacc.Bacc):
        """Restrict the activation-table-load pass to one set holding
        Ln+Exp+Copy so alternating Ln/Exp doesn't reload tables every tile."""

        def insert_act_table_loads(self):
            import bass_rust
            import concourse.mybir as mybir
            from concourse.hw_specs import get_activation_tables

            has_activation = any(
                isinstance(i, mybir.InstActivation)
                for b in self.main_func.blocks
                for i in b.instructions
            )
            if not has_activation:
                return
            tables = [
                (name, fns if name == _COMBINED_SET else set())
                for name, fns in get_activation_tables(self.m.arch).items()
            ]
            bass_rust.insert_act_table_loads(self, tables)

    return SingleActSetBacc


# --------------------------------------------------------------------------
# Hand-written 2X_1PORT pair-scan uop programs (scaled variants).
# --------------------------------------------------------------------------

def _mk_pv2xs_uops():
    """pv = cumsum(x*(k*C)) with C = CONST_0 (s0 immediate), one packed fp16
    pair/cycle:
      k'_e = k_e*C; k'_o = k_o*C; m_e = x_e*k'_e; m_o = x_o*k'_o;
      p = m_o + m_e; z += p; y_e = z - m_o; y_o = z."""
    from concourse.dve_uop import (
        ENABLE, AluInp, AluOp, DelayInp, InpSel, OutPath, OutSel,
        Trigger, UopConfig,
    )

    def base_inputs(u):
        u.enable_input(InpSel.SRC_0, 0)        # x_e -> stage0 ALU A
        u.enable_input(InpSel.SRC_1, 1)        # k_e -> PREV_DELAY_0
        u.enable_input(InpSel.SRC_0_HI, 2)     # x_o -> PREV_DELAY_1
        u.enable_input(InpSel.SRC_1_HI, 3)     # k_o -> PREV_DELAY_2
        u.enable_input(InpSel.CONST_0, 4)      # C   -> PREV_DELAY_3
        u.enable_input(InpSel.ZERO, 5)         # 0   -> PREV_DELAY_4

    # seed uop: one dummy cycle seeds stage5's CURR_ALU_OUT (z) with 0
    seed = UopConfig()
    base_inputs(seed)
    seed.repeat_count = 1
    seed.trigger = (Trigger.COUNT, Trigger.NONE, Trigger.NONE)
    seed.next_uop = (1, 0, 0)
    d = seed.datapath_config
    for k in range(5):
        d[k].pass_through_alu()
        d[k].pass_through_delay(4)             # carry ZERO to stage 5
    d[5].enable_alu(AluOp.BYPASS, AluInp.PREV_DELAY_4)   # z flop <- 0
    for k in range(6, 8):
        d[k].pass_through_alu()

    st = UopConfig()
    base_inputs(st)
    st.require_inp0 = ENABLE
    st.require_inp1 = ENABLE
    st.trigger = (Trigger.SRC_TENSOR_DONE, Trigger.NONE, Trigger.NONE)
    st.next_uop = (0, 0, 0)
    st.enable_output(OutSel.ALU_OUT, OutPath.WR0_LO)     # y_even
    st.enable_output(OutSel.DELAY_0, OutPath.WR0_HI)     # y_odd = z
    d = st.datapath_config
    d[0].enable_alu(AluOp.MULTIPLY, AluInp.PREV_DELAY_0, AluInp.PREV_DELAY_3)
    d[0].enable_delay_from_src(DelayInp.PREV_ALU_OUT, 0)  # capture x_e
    d[0].pass_through_delay(1, 2, 3)           # x_o, k_o, C
    d[1].enable_alu(AluOp.MULTIPLY, AluInp.PREV_DELAY_2, AluInp.PREV_DELAY_3)
    d[1].pass_through_delay(0, 1)              # x_e, x_o
    d[1].enable_delay_from_src(DelayInp.PREV_ALU_OUT, 3)  # capture k'_e
    d[2].enable_alu(AluOp.MULTIPLY, AluInp.PREV_DELAY_0, AluInp.PREV_DELAY_3)
    d[2].pass_through_delay(1)                 # x_o
    d[2].enable_delay_from_src(DelayInp.PREV_ALU_OUT, 2)  # capture k'_o
    d[3].enable_alu(AluOp.MULTIPLY, AluInp.PREV_DELAY_1, AluInp.PREV_DELAY_2)
    d[3].enable_delay_from_src(DelayInp.PREV_ALU_OUT, 3)  # capture m_e
    d[4].enable_alu(AluOp.ADD, AluInp.PREV_ALU_OUT, AluInp.PREV_DELAY_3)  # p
    d[4].enable_delay_from_src(DelayInp.PREV_ALU_OUT, 1)  # capture m_o
    d[5].enable_alu(AluOp.ADD, AluInp.CURR_ALU_OUT, AluInp.PREV_ALU_OUT)  # z
    d[5].pass_through_delay(1)                 # m_o
    d[6].enable_alu(AluOp.SUBTRACT, AluInp.PREV_ALU_OUT, AluInp.PREV_DELAY_1)
    d[6].enable_delay_from_src(DelayInp.PREV_ALU_OUT, 0)  # capture z
    d[7].pass_through_alu()
    d[7].pass_through_delay(0)
    return [seed, st]


def _mk_v2xs_uops():
    """vc = cumsum(k*C) pair-scan; SRC_1 is required and drained (keeps the
    op in the TwoSrc perf class) but ignored."""
    from concourse.dve_uop import (
        ENABLE, AluInp, AluOp, DelayInp, InpSel, OutPath, OutSel,
        Trigger, UopConfig,
    )

    def base_inputs(u):
        u.enable_input(InpSel.SRC_0, 0)        # k_e -> stage0 ALU A
        u.enable_input(InpSel.SRC_1, 1)        # drained, value ignored
        u.enable_input(InpSel.SRC_0_HI, 2)     # k_o -> PREV_DELAY_1
        u.enable_input(InpSel.CONST_0, 3)      # C   -> PREV_DELAY_2
        u.enable_input(InpSel.ZERO, 4)         # 0   -> PREV_DELAY_3

    seed = UopConfig()
    base_inputs(seed)
    seed.repeat_count = 1
    seed.trigger = (Trigger.COUNT, Trigger.NONE, Trigger.NONE)
    seed.next_uop = (1, 0, 0)
    d = seed.datapath_config
    for k in range(3):
        d[k].pass_through_alu()
        d[k].pass_through_delay(3)
    d[3].enable_alu(AluOp.BYPASS, AluInp.PREV_DELAY_3)   # w flop <- 0
    for k in range(4, 8):
        d[k].pass_through_alu()

    st = UopConfig()
    base_inputs(st)
    st.require_inp0 = ENABLE
    st.require_inp1 = ENABLE
    st.trigger = (Trigger.SRC_TENSOR_DONE, Trigger.NONE, Trigger.NONE)
    st.next_uop = (0, 0, 0)
    st.enable_output(OutSel.ALU_OUT, OutPath.WR0_LO)     # y_even
    st.enable_output(OutSel.DELAY_0, OutPath.WR0_HI)     # y_odd = w
    d = st.datapath_config
    d[0].enable_alu(AluOp.MULTIPLY, AluInp.PREV_ALU_OUT, AluInp.PREV_DELAY_2)
    d[0].pass_through_delay(1, 2)              # k_o, C
    d[1].enable_alu(AluOp.MULTIPLY, AluInp.PREV_DELAY_1, AluInp.PREV_DELAY_2)
    d[1].enable_delay_from_src(DelayInp.PREV_ALU_OUT, 0)  # capture k'_e
    d[2].enable_alu(AluOp.ADD, AluInp.PREV_ALU_OUT, AluInp.PREV_DELAY_0)  # q
    d[2].enable_delay_from_src(DelayInp.PREV_ALU_OUT, 1)  # capture k'_o
    d[3].enable_alu(AluOp.ADD, AluInp.CURR_ALU_OUT, AluInp.PREV_ALU_OUT)  # w
    d[3].pass_through_delay(1)                 # k'_o
    d[4].enable_alu(AluOp.SUBTRACT, AluInp.PREV_ALU_OUT, AluInp.PREV_DELAY_1)
    d[4].enable_delay_from_src(DelayInp.PREV_ALU_OUT, 0)  # capture w
    for k in range(5, 8):
        d[k].pass_through_alu()
        d[k].pass_through_delay(0)
    return [seed, st]


def _register_pair_ops():
    """Register the scaled pair-scan DveOps; the hand 2x program is injected
    via the compile cache so DveOp.compile returns it table-generation-time."""
    import concourse.dve_ops as dve_ops
    from concourse.dve_ops import DveOp, _COMPILE_CACHE
    from concourse.dve_spec import AluOp, C0, Spec, Src0, Src1, lower, scan
    from concourse.dve_uop import DveOpSpec

    def mk(name, spec, uops_2x):
        for o in dve_ops.OPS:
            if o.name == name:
                return o
        op = DveOp(name, spec, subdim=False, uops_sha={})
        dve_ops.OPS.append(op)
        dve_ops.CUSTOM_DVE_SPECS[name] = spec
        dve_ops._SUB_OPCODE_FOR_NAME[name] = (
            dve_ops._CUSTOM_DVE_ROW_BASE + len(dve_ops.OPS) - 1
        )
        assert dve_ops._SUB_OPCODE_FOR_NAME[name] < 0x20
        for ver in ("v3", "v4"):
            s = DveOpSpec(
                name=name,
                opcode=dve_ops.get_dve_sub_opcode(name),
                uops=lower(spec, ver=ver),
                uops_2x=uops_2x,
                perf_max=1,
                rd1_en=True,
            )
            op.uops_sha[ver] = s.sha(ver)
            _COMPILE_CACHE[(name, ver)] = s
        return op

    pv = mk(
        "PV2XS_ANT",
        Spec(
            body=scan(AluOp.ADD, Src0 * (Src1 * C0)),
            reference=lambda in0, in1, s0, s1, imm2: np.cumsum(
                in0.astype(np.float32) * (in1.astype(np.float32) * np.float32(s0)),
                axis=-1, dtype=np.float32,
            ),
        ),
        _mk_pv2xs_uops(),
    )
    v = mk(
        "V2XS_ANT",
        Spec(
            body=scan(AluOp.ADD, Src0 * C0),
            reference=lambda in0, in1, s0, s1, imm2: np.cumsum(
                in0.astype(np.float32) * np.float32(s0), axis=-1,
                dtype=np.float32,
            ),
        ),
        _mk_v2xs_uops(),
    )
    return pv, v


def _build(rows=ROWS, time=TIME, bufs=3, reps=1):
    import concourse.tile as tile
    import concourse.mybir as mybir

    pv_op, v_op = _register_pair_ops()
    nc = _single_act_set_bacc()("TRN2", target_bir_lowering=False, debug=False)
    f32 = mybir.dt.float32
    f16 = mybir.dt.float16
    u8 = mybir.dt.uint8
    x8d = nc.dram_tensor("stock_codes", [rows, time], u8, kind="ExternalInput").ap()
    k8d = nc.dram_tensor("volume_codes", [rows, time], u8, kind="ExternalInput").ap()
    out = nc.dram_tensor("vwap_f16", [rows, time], f16, kind="ExternalOutput").ap()

    Ln = mybir.ActivationFunctionType.Ln
    Exp = mybir.ActivationFunctionType.Exp

    n_tiles = rows // P
    total = n_tiles * reps
    with tile.TileContext(nc) as tc:
        with tc.tile_pool(name="big", bufs=bufs) as big:
            pend = {}

            def issue_loads(j):
                r0 = (j % n_tiles) * P
                xt = big.tile([P, time], f16, tag="xt")
                kt = big.tile([P, time], f16, tag="kt")
                nc.gpsimd.dma_start(xt[:], x8d[r0 : r0 + P, :])
                nc.gpsimd.dma_start(kt[:], k8d[r0 : r0 + P, :])
                pend[j] = (xt, kt)

            for j in range(min(2, total)):
                issue_loads(j)
            for i in range(total):
                if i + 2 < total:
                    issue_loads(i + 2)
                xt, kt = pend.pop(i)
                r0 = (i % n_tiles) * P
                pv = big.tile([P, time], f16, tag="pv")
                vc = big.tile([P, time], f16, tag="vc")
                ln = big.tile([P, time], f32, tag="ln")
                oc = big.tile([P, time], f16, tag="oc")
                i1 = nc.vector._custom_dve(
                    pv_op, out=pv[:], in0=xt[:], in1=kt[:], s0=C_PV
                )
                i2 = nc.vector._custom_dve(
                    v_op, out=vc[:], in0=kt[:], in1=xt[:], s0=C_V
                )
                i1.ins.perf_max = 1
                i2.ins.perf_max = 1
                nc.scalar.activation(ln[:], vc[:], Ln, scale=C_LN)
                nc.scalar.activation(vc[:], ln[:], Exp, scale=-1.0)
                nc.vector.tensor_mul(oc[:], pv[:], vc[:])
                nc.scalar.dma_start(out[r0 : r0 + P, :], oc[:])
    nc.compile()
    return nc


def _get_nc():
    if "nc" not in _CACHE:
        _CACHE["nc"] = _build()
    return _CACHE["nc"]


def _prep_inputs(stock_paths, volume_paths):
    x8 = np.clip(
        np.round((stock_paths - np.float32(30.0)) * np.float32(255.0 / 30.0)),
        0, 255,
    ).astype(np.uint8)
    k8 = np.clip(
        np.round(volume_paths * np.float32(255.0 / 8e6)), 0, 255
    ).astype(np.uint8)
    return x8, k8


def kernel(stock_paths: np.ndarray, volume_paths: np.ndarray) -> np.ndarray:
    from concourse.bass_utils import run_bass_kernel_spmd

    stock_paths = np.ascontiguousarray(stock_paths, dtype=np.float32)
    volume_paths = np.ascontiguousarray(volume_paths, dtype=np.float32)
    assert stock_paths.shape == (NUM_PATHS, TIME)

    x8, k8 = _prep_inputs(stock_paths, volume_paths)
    nc = _get_nc()
    in_maps = [
        {
            "stock_codes": x8[i * ROWS : (i + 1) * ROWS],
            "volume_codes": k8[i * ROWS : (i + 1) * ROWS],
        }
        for i in range(N_CORES)
    ]
    res = run_bass_kernel_spmd(nc, in_maps, core_ids=list(range(N_CORES)))
    return np.concatenate(
        [
            r["vwap_f16"].astype(np.float32) * np.float32(30.0 / 255.0)
            + np.float32(30.0)
            for r in res.results
        ],
        axis=0,
    )


# revision 9
# speedup vs baseline: 9.9146x; 9.9146x over previous
"""Cumulative VWAP kernel for Trainium2 (Bass/Tile), data-parallel over 8 cores.

vwap[:, t] = cumsum(s*v)[:, t] / (cumsum(v)[:, t] + 1e-8),  vwap[:, 0] = s[:, 0]

Sharding: num_paths (axis 0) split evenly across 8 NeuronCores; the cumsum
runs along the time axis, which stays local to each core (no collectives).

v2: u8-coded I/O.  The problem is HBM-bound; the rel-err budget (2e-2) is
spent on 8-bit storage codes instead of fp16:
  * stock  -> x8 = round((s-30)*255/30)   (affine code; s in [30,60))
  * volume -> k8 = round(v*255/8e6)       (pure-scale code; v in [1e6,8e6))
  * output -> o8 with vwap = o8*(30/255)+30
The affine offsets cancel in the VWAP algebra: with s-30 = (30/255)*x and
v = (8e6/255)*k,  (vwap-30)*255/30 = cumsum(x*k)/cumsum(k), so the device
works on raw codes only.  Measured end-to-end rel err ~3.8e-3 (the per-
element quantization averages out in the sums) -- 3x better than the fp16
baseline while moving half the bytes.

Loads are SWDGE (gpsimd) cast-DMAs (u8 HBM -> fp16 SBUF); the store is a
plain fp16 HWDGE DMA on the otherwise-idle SP queue (the ACT queue would
head-of-line block Ln/Exp behind the store's mul dependency, and a u8 cast
store must share the single SWDGE ring with the loads).  2 MiB/tile HBM-side
vs 3 MiB for the fp16 baseline; more importantly the binding resource is the
shared SBUF AXI fabric (~435 GB/s, measured 431 aggregate): 3 MiB/tile SBUF-
side = 7.2 us/tile = 116 us/core floor, and the kernel runs ~119 us/core.
(Cast loads are SBUF-side bound: the u8 HBM savings mostly vanish at the
fabric; raw-u8 loads would need an on-chip int->fp decode no engine can do
at rate, and a u8-out mul drops the DVE to 1x -- both measured dead ends.)

The two cumsums run as hand-written custom-DVE uop programs in the 2X_1PORT
perf mode (2 fp16 code pairs/cycle), with a power-of-2 scale (CONST_0 <-
s0 immediate) folded into the scan so the running sums fit fp16:
  PV2XS: pv = cumsum(x*k)*2^-12   (fp16, max |pv| < 2^16 ok)
  V2XS : vc = cumsum(k)*2^-5      (fp16)
The 2^17 net scale plus the output code scale folds into the Exp bias:
  r = exp(-ln(vc) + 7*ln2) = 2^7/vc;  oc = pv*r = (vwap-30)*255/30.
oc[:,0] = x_0*(1 +- 6e-4) rounds to exactly x_0 in the u8 store, so no
explicit t==0 column fix is needed.

Per-core dataflow, per [128, 4096] tile (16 tiles per core):
  DMA(gpsimd): x8 -> xt fp16, k8 -> kt fp16 (cast loads, issued 2 tiles
               ahead of the store in the Pool queue to dodge head-of-line
               blocking on the store's data dependency)
  DVE : pv = PV2XS(xt, kt)        fp16, ~2.2 us
  DVE : vc = V2XS(kt, xt)         fp16 (xt drained; keeps TwoSrc class)
  ACT : ln = Ln(vc)               f32
  ACT : r  = Exp(-ln + 7ln2)      fp16 (in-place into vc's buffer)
  DVE : oc = pv * r               fp16 tensor_tensor 2x
  DMA(sync/SP): oc -> vwap_f16 (plain fp16 store)
Host decode: vwap = vwap_f16 * 30/255 + 30.
"""

import numpy as np

NUM_PATHS = 16384
TIME = 4096
N_CORES = 8
ROWS = NUM_PATHS // N_CORES  # rows per core
P = 128  # SBUF partitions

C_PV = float(2.0 ** -12)
C_V = float(2.0 ** -5)
C_LN = float(2.0 ** -7)

_CACHE = {}

_COMBINED_SET = "natural_log_exp_and_others"


def _single_act_set_bacc():
    import concourse.bacc as bacc

    class SingleActSetBacc(bacc.Bacc):
        """Restrict the activation-table-load pass to one set holding
        Ln+Exp+Copy so alternating Ln/Exp doesn't reload tables every tile."""

        def insert_act_table_loads(self):
            import bass_rust
            import concourse.mybir as mybir
            from concourse.hw_specs import get_activation_tables

            has_activation = any(
                isinstance(i, mybir.InstActivation)
                for b in self.main_func.blocks
                for i in b.instructions
            )
            if not has_activation:
                return
            tables = [
                (name, fns if name == _COMBINED_SET else set())
                for name, fns in get_activation_tables(self.m.arch).items()
            ]
            bass_rust.insert_act_table_loads(self, tables)

    return SingleActSetBacc


# --------------------------------------------------------------------------
# Hand-written 2X_1PORT pair-scan uop programs (scaled variants).
# --------------------------------------------------------------------------

def _mk_pv2xs_uops():
    """pv = cumsum(x*(k*C)) with C = CONST_0 (s0 immediate), one packed fp16
    pair/cycle:
      k'_e = k_e*C; k'_o = k_o*C; m_e = x_e*k'_e; m_o = x_o*k'_o;
      p = m_o + m_e; z += p; y_e = z - m_o; y_o = z."""
    from concourse.dve_uop import (
        ENABLE, AluInp, AluOp, DelayInp, InpSel, OutPath, OutSel,
        Trigger, UopConfig,
    )

    def base_inputs(u):
        u.enable_input(InpSel.SRC_0, 0)        # x_e -> stage0 ALU A
        u.enable_input(InpSel.SRC_1, 1)        # k_e -> PREV_DELAY_0
        u.enable_input(InpSel.SRC_0_HI, 2)     # x_o -> PREV_DELAY_1
        u.enable_input(InpSel.SRC_1_HI, 3)     # k_o -> PREV_DELAY_2
        u.enable_input(InpSel.CONST_0, 4)      # C   -> PREV_DELAY_3
        u.enable_input(InpSel.ZERO, 5)         # 0   -> PREV_DELAY_4

    # seed uop: one dummy cycle seeds stage5's CURR_ALU_OUT (z) with 0
    seed = UopConfig()
    base_inputs(seed)
    seed.repeat_count = 1
    seed.trigger = (Trigger.COUNT, Trigger.NONE, Trigger.NONE)
    seed.next_uop = (1, 0, 0)
    d = seed.datapath_config
    for k in range(5):
        d[k].pass_through_alu()
        d[k].pass_through_delay(4)             # carry ZERO to stage 5
    d[5].enable_alu(AluOp.BYPASS, AluInp.PREV_DELAY_4)   # z flop <- 0
    for k in range(6, 8):
        d[k].pass_through_alu()

    st = UopConfig()
    base_inputs(st)
    st.require_inp0 = ENABLE
    st.require_inp1 = ENABLE
    st.trigger = (Trigger.SRC_TENSOR_DONE, Trigger.NONE, Trigger.NONE)
    st.next_uop = (0, 0, 0)
    st.enable_output(OutSel.ALU_OUT, OutPath.WR0_LO)     # y_even
    st.enable_output(OutSel.DELAY_0, OutPath.WR0_HI)     # y_odd = z
    d = st.datapath_config
    d[0].enable_alu(AluOp.MULTIPLY, AluInp.PREV_DELAY_0, AluInp.PREV_DELAY_3)
    d[0].enable_delay_from_src(DelayInp.PREV_ALU_OUT, 0)  # capture x_e
    d[0].pass_through_delay(1, 2, 3)           # x_o, k_o, C
    d[1].enable_alu(AluOp.MULTIPLY, AluInp.PREV_DELAY_2, AluInp.PREV_DELAY_3)
    d[1].pass_through_delay(0, 1)              # x_e, x_o
    d[1].enable_delay_from_src(DelayInp.PREV_ALU_OUT, 3)  # capture k'_e
    d[2].enable_alu(AluOp.MULTIPLY, AluInp.PREV_DELAY_0, AluInp.PREV_DELAY_3)
    d[2].pass_through_delay(1)                 # x_o
    d[2].enable_delay_from_src(DelayInp.PREV_ALU_OUT, 2)  # capture k'_o
    d[3].enable_alu(AluOp.MULTIPLY, AluInp.PREV_DELAY_1, AluInp.PREV_DELAY_2)
    d[3].enable_delay_from_src(DelayInp.PREV_ALU_OUT, 3)  # capture m_e
    d[4].enable_alu(AluOp.ADD, AluInp.PREV_ALU_OUT, AluInp.PREV_DELAY_3)  # p
    d[4].enable_delay_from_src(DelayInp.PREV_ALU_OUT, 1)  # capture m_o
    d[5].enable_alu(AluOp.ADD, AluInp.CURR_ALU_OUT, AluInp.PREV_ALU_OUT)  # z
    d[5].pass_through_delay(1)                 # m_o
    d[6].enable_alu(AluOp.SUBTRACT, AluInp.PREV_ALU_OUT, AluInp.PREV_DELAY_1)
    d[6].enable_delay_from_src(DelayInp.PREV_ALU_OUT, 0)  # capture z
    d[7].pass_through_alu()
    d[7].pass_through_delay(0)
    return [seed, st]


def _mk_v2xs_uops():
    """vc = cumsum(k*C) pair-scan; SRC_1 is required and drained (keeps the
    op in the TwoSrc perf class) but ignored."""
    from concourse.dve_uop import (
        ENABLE, AluInp, AluOp, DelayInp, InpSel, OutPath, OutSel,
        Trigger, UopConfig,
    )

    def base_inputs(u):
        u.enable_input(InpSel.SRC_0, 0)        # k_e -> stage0 ALU A
        u.enable_input(InpSel.SRC_1, 1)        # drained, value ignored
        u.enable_input(InpSel.SRC_0_HI, 2)     # k_o -> PREV_DELAY_1
        u.enable_input(InpSel.CONST_0, 3)      # C   -> PREV_DELAY_2
        u.enable_input(InpSel.ZERO, 4)         # 0   -> PREV_DELAY_3

    seed = UopConfig()
    base_inputs(seed)
    seed.repeat_count = 1
    seed.trigger = (Trigger.COUNT, Trigger.NONE, Trigger.NONE)
    seed.next_uop = (1, 0, 0)
    d = seed.datapath_config
    for k in range(3):
        d[k].pass_through_alu()
        d[k].pass_through_delay(3)
    d[3].enable_alu(AluOp.BYPASS, AluInp.PREV_DELAY_3)   # w flop <- 0
    for k in range(4, 8):
        d[k].pass_through_alu()

    st = UopConfig()
    base_inputs(st)
    st.require_inp0 = ENABLE
    st.require_inp1 = ENABLE
    st.trigger = (Trigger.SRC_TENSOR_DONE, Trigger.NONE, Trigger.NONE)
    st.next_uop = (0, 0, 0)
    st.enable_output(OutSel.ALU_OUT, OutPath.WR0_LO)     # y_even
    st.enable_output(OutSel.DELAY_0, OutPath.WR0_HI)     # y_odd = w
    d = st.datapath_config
    d[0].enable_alu(AluOp.MULTIPLY, AluInp.PREV_ALU_OUT, AluInp.PREV_DELAY_2)
    d[0].pass_through_delay(1, 2)              # k_o, C
    d[1].enable_alu(AluOp.MULTIPLY, AluInp.PREV_DELAY_1, AluInp.PREV_DELAY_2)
    d[1].enable_delay_from_src(DelayInp.PREV_ALU_OUT, 0)  # capture k'_e
    d[2].enable_alu(AluOp.ADD, AluInp.PREV_ALU_OUT, AluInp.PREV_DELAY_0)  # q
    d[2].enable_delay_from_src(DelayInp.PREV_ALU_OUT, 1)  # capture k'_o
    d[3].enable_alu(AluOp.ADD, AluInp.CURR_ALU_OUT, AluInp.PREV_ALU_OUT)  # w
    d[3].pass_through_delay(1)                 # k'_o
    d[4].enable_alu(AluOp.SUBTRACT, AluInp.PREV_ALU_OUT, AluInp.PREV_DELAY_1)
    d[4].enable_delay_from_src(DelayInp.PREV_ALU_OUT, 0)  # capture w
    for k in range(5, 8):
        d[k].pass_through_alu()
        d[k].pass_through_delay(0)
    return [seed, st]


def _register_pair_ops():
    """Register the scaled pair-scan DveOps; the hand 2x program is injected
    via the compile cache so DveOp.compile returns it table-generation-time."""
    import concourse.dve_ops as dve_ops
    from concourse.dve_ops import DveOp, _COMPILE_CACHE
    from concourse.dve_spec import AluOp, C0, Spec, Src0, Src1, lower, scan
    from concourse.dve_uop import DveOpSpec

    def mk(name, spec, uops_2x):
        for o in dve_ops.OPS:
            if o.name == name:
                return o
        op = DveOp(name, spec, subdim=False, uops_sha={})
        dve_ops.OPS.append(op)
        dve_ops.CUSTOM_DVE_SPECS[name] = spec
        dve_ops._SUB_OPCODE_FOR_NAME[name] = (
            dve_ops._CUSTOM_DVE_ROW_BASE + len(dve_ops.OPS) - 1
        )
        assert dve_ops._SUB_OPCODE_FOR_NAME[name] < 0x20
        for ver in ("v3", "v4"):
            s = DveOpSpec(
                name=name,
                opcode=dve_ops.get_dve_sub_opcode(name),
                uops=lower(spec, ver=ver),
                uops_2x=uops_2x,
                perf_max=1,
                rd1_en=True,
            )
            op.uops_sha[ver] = s.sha(ver)
            _COMPILE_CACHE[(name, ver)] = s
        return op

    pv = mk(
        "PV2XS_ANT",
        Spec(
            body=scan(AluOp.ADD, Src0 * (Src1 * C0)),
            reference=lambda in0, in1, s0, s1, imm2: np.cumsum(
                in0.astype(np.float32) * (in1.astype(np.float32) * np.float32(s0)),
                axis=-1, dtype=np.float32,
            ),
        ),
        _mk_pv2xs_uops(),
    )
    v = mk(
        "V2XS_ANT",
        Spec(
            body=scan(AluOp.ADD, Src0 * C0),
            reference=lambda in0, in1, s0, s1, imm2: np.cumsum(
                in0.astype(np.float32) * np.float32(s0), axis=-1,
                dtype=np.float32,
            ),
        ),
        _mk_v2xs_uops(),
    )
    return pv, v


def _build(rows=ROWS, time=TIME, bufs=3, reps=1):
    import concourse.tile as tile
    import concourse.mybir as mybir

    pv_op, v_op = _register_pair_ops()
    nc = _single_act_set_bacc()("TRN2", target_bir_lowering=False, debug=False)
    f32 = mybir.dt.float32
    f16 = mybir.dt.float16
    u8 = mybir.dt.uint8
    x8d = nc.dram_tensor("stock_codes", [rows, time], u8, kind="ExternalInput").ap()
    k8d = nc.dram_tensor("volume_codes", [rows, time], u8, kind="ExternalInput").ap()
    out = nc.dram_tensor("vwap_f16", [rows, time], f16, kind="ExternalOutput").ap()

    Ln = mybir.ActivationFunctionType.Ln
    Exp = mybir.ActivationFunctionType.Exp

    n_tiles = rows // P
    total = n_tiles * reps
    LOOKAHEAD = 2
    with tile.TileContext(nc) as tc:
        with tc.tile_pool(name="big", bufs=bufs) as big:
            pend = {}

            def issue_loads(j):
                r0 = (j % n_tiles) * P
                xt = big.tile([P, time], f16, tag="xt")
                kt = big.tile([P, time], f16, tag="kt")
                nc.gpsimd.dma_start(xt[:], x8d[r0 : r0 + P, :])
                nc.gpsimd.dma_start(kt[:], k8d[r0 : r0 + P, :])
                pend[j] = (xt, kt)

            for j in range(min(LOOKAHEAD, total)):
                issue_loads(j)
            for i in range(total):
                if i + LOOKAHEAD < total:
                    issue_loads(i + LOOKAHEAD)
                xt, kt = pend.pop(i)
                r0 = (i % n_tiles) * P
                pv = big.tile([P, time], f16, tag="pv")
                vc = big.tile([P, time], f16, tag="vc")
                ln = big.tile([P, time], f32, tag="ln")
                oc = big.tile([P, time], f16, tag="oc")
                i1 = nc.vector._custom_dve(
                    pv_op, out=pv[:], in0=xt[:], in1=kt[:], s0=C_PV
                )
                i2 = nc.vector._custom_dve(
                    v_op, out=vc[:], in0=kt[:], in1=xt[:], s0=C_V
                )
                i1.ins.perf_max = 1
                i2.ins.perf_max = 1
                nc.scalar.activation(ln[:], vc[:], Ln, scale=C_LN)
                nc.scalar.activation(vc[:], ln[:], Exp, scale=-1.0)
                nc.vector.tensor_mul(oc[:], pv[:], vc[:])
                nc.sync.dma_start(out[r0 : r0 + P, :], oc[:])
    nc.compile()
    return nc


def _get_nc():
    if "nc" not in _CACHE:
        _CACHE["nc"] = _build()
    return _CACHE["nc"]


def _prep_inputs(stock_paths, volume_paths):
    x8 = np.clip(
        np.round((stock_paths - np.float32(30.0)) * np.float32(255.0 / 30.0)),
        0, 255,
    ).astype(np.uint8)
    k8 = np.clip(
        np.round(volume_paths * np.float32(255.0 / 8e6)), 0, 255
    ).astype(np.uint8)
    return x8, k8


def kernel(stock_paths: np.ndarray, volume_paths: np.ndarray) -> np.ndarray:
    from concourse.bass_utils import run_bass_kernel_spmd

    stock_paths = np.ascontiguousarray(stock_paths, dtype=np.float32)
    volume_paths = np.ascontiguousarray(volume_paths, dtype=np.float32)
    assert stock_paths.shape == (NUM_PATHS, TIME)

    x8, k8 = _prep_inputs(stock_paths, volume_paths)
    nc = _get_nc()
    in_maps = [
        {
            "stock_codes": x8[i * ROWS : (i + 1) * ROWS],
            "volume_codes": k8[i * ROWS : (i + 1) * ROWS],
        }
        for i in range(N_CORES)
    ]
    res = run_bass_kernel_spmd(nc, in_maps, core_ids=list(range(N_CORES)))
    return np.concatenate(
        [
            r["vwap_f16"].astype(np.float32) * np.float32(30.0 / 255.0)
            + np.float32(30.0)
            for r in res.results
        ],
        axis=0,
    )


# revision 13
# speedup vs baseline: 10.2860x; 1.0375x over previous
"""Cumulative VWAP kernel for Trainium2 (Bass/Tile), data-parallel over 8 cores.

vwap[:, t] = cumsum(s*v)[:, t] / (cumsum(v)[:, t] + 1e-8),  vwap[:, 0] = s[:, 0]

Sharding: num_paths (axis 0) split evenly across 8 NeuronCores; the cumsum
runs along the time axis, which stays local to each core (no collectives).

v2: u8-coded I/O.  The problem is HBM-bound; the rel-err budget (2e-2) is
spent on 8-bit storage codes instead of fp16:
  * stock  -> x8 = round((s-30)*255/30)   (affine code; s in [30,60))
  * volume -> k8 = round(v*255/8e6)       (pure-scale code; v in [1e6,8e6))
  * output -> o8 with vwap = o8*(30/255)+30
The affine offsets cancel in the VWAP algebra: with s-30 = (30/255)*x and
v = (8e6/255)*k,  (vwap-30)*255/30 = cumsum(x*k)/cumsum(k), so the device
works on raw codes only.  Measured end-to-end rel err ~3.8e-3 (the per-
element quantization averages out in the sums) -- 3x better than the fp16
baseline while moving half the bytes.

Loads are SWDGE (gpsimd) cast-DMAs (u8 HBM -> fp16 SBUF); the store is a
plain fp16 HWDGE DMA on the otherwise-idle SP queue (the ACT queue would
head-of-line block Ln/Exp behind the store's mul dependency, and a u8 cast
store must share the single SWDGE ring with the loads).  2 MiB/tile HBM-side
vs 3 MiB for the fp16 baseline; more importantly the binding resource is the
shared SBUF AXI fabric (~435 GB/s, measured 431 aggregate): 3 MiB/tile SBUF-
side = 7.2 us/tile = 116 us/core floor, and the kernel runs ~119 us/core.
(Cast loads are SBUF-side bound: the u8 HBM savings mostly vanish at the
fabric; raw-u8 loads would need an on-chip int->fp decode no engine can do
at rate, and a u8-out mul drops the DVE to 1x -- both measured dead ends.)

The two cumsums run as hand-written custom-DVE uop programs in the 2X_1PORT
perf mode (2 fp16 code pairs/cycle), with a power-of-2 scale (CONST_0 <-
s0 immediate) folded into the scan so the running sums fit fp16:
  PV2XS: pv = cumsum(x*k)*2^-12   (fp16, max |pv| < 2^16 ok)
  V2XS : vc = cumsum(k)*2^-5      (fp16)
The 2^17 net scale plus the output code scale folds into the Exp bias:
  r = exp(-ln(vc) + 7*ln2) = 2^7/vc;  oc = pv*r = (vwap-30)*255/30.
oc[:,0] = x_0*(1 +- 6e-4) rounds to exactly x_0 in the u8 store, so no
explicit t==0 column fix is needed.

Per-core dataflow, per [128, 4096] tile (16 tiles per core):
  DMA(gpsimd): x8 -> xt fp16, k8 -> kt fp16 (cast loads, issued 2 tiles
               ahead of the store in the Pool queue to dodge head-of-line
               blocking on the store's data dependency)
  DVE : pv = PV2XS(xt, kt)        fp16, ~2.2 us
  DVE : vc = V2XS(kt, xt)         fp16 (xt drained; keeps TwoSrc class)
  ACT : ln = Ln(vc)               f32
  ACT : r  = Exp(-ln + 7ln2)      fp16 (in-place into vc's buffer)
  DVE : oc = pv * r               fp16 tensor_tensor 2x
  DMA(sync/SP): oc -> vwap_f16 (plain fp16 store)
Host decode: vwap = vwap_f16 * 30/255 + 30.
"""

import numpy as np

NUM_PATHS = 16384
TIME = 4096
N_CORES = 8
ROWS = NUM_PATHS // N_CORES  # rows per core
P = 128  # SBUF partitions

C_PV = float(2.0 ** -12)
C_V = float(2.0 ** -5)
C_LN = float(2.0 ** -7)

_CACHE = {}

_COMBINED_SET = "natural_log_exp_and_others"


def _single_act_set_bacc():
    import concourse.bacc as bacc

    class SingleActSetBacc(bacc.Bacc):
        """Restrict the activation-table-load pass to one set holding
        Ln+Exp+Copy so alternating Ln/Exp doesn't reload tables every tile."""

        def insert_act_table_loads(self):
            import bass_rust
            import concourse.mybir as mybir
            from concourse.hw_specs import get_activation_tables

            has_activation = any(
                isinstance(i, mybir.InstActivation)
                for b in self.main_func.blocks
                for i in b.instructions
            )
            if not has_activation:
                return
            tables = [
                (name, fns if name == _COMBINED_SET else set())
                for name, fns in get_activation_tables(self.m.arch).items()
            ]
            bass_rust.insert_act_table_loads(self, tables)

    return SingleActSetBacc


# --------------------------------------------------------------------------
# Hand-written 2X_1PORT pair-scan uop programs (scaled variants).
# --------------------------------------------------------------------------

def _mk_pv2xs_uops():
    """pv = cumsum(x*(k*C)) with C = CONST_0 (s0 immediate), one packed fp16
    pair/cycle:
      k'_e = k_e*C; k'_o = k_o*C; m_e = x_e*k'_e; m_o = x_o*k'_o;
      p = m_o + m_e; z += p; y_e = z - m_o; y_o = z."""
    from concourse.dve_uop import (
        ENABLE, AluInp, AluOp, DelayInp, InpSel, OutPath, OutSel,
        Trigger, UopConfig,
    )

    def base_inputs(u):
        u.enable_input(InpSel.SRC_0, 0)        # x_e -> stage0 ALU A
        u.enable_input(InpSel.SRC_1, 1)        # k_e -> PREV_DELAY_0
        u.enable_input(InpSel.SRC_0_HI, 2)     # x_o -> PREV_DELAY_1
        u.enable_input(InpSel.SRC_1_HI, 3)     # k_o -> PREV_DELAY_2
        u.enable_input(InpSel.CONST_0, 4)      # C   -> PREV_DELAY_3
        u.enable_input(InpSel.ZERO, 5)         # 0   -> PREV_DELAY_4

    # seed uop: one dummy cycle seeds stage5's CURR_ALU_OUT (z) with 0
    seed = UopConfig()
    base_inputs(seed)
    seed.repeat_count = 1
    seed.trigger = (Trigger.COUNT, Trigger.NONE, Trigger.NONE)
    seed.next_uop = (1, 0, 0)
    d = seed.datapath_config
    for k in range(5):
        d[k].pass_through_alu()
        d[k].pass_through_delay(4)             # carry ZERO to stage 5
    d[5].enable_alu(AluOp.BYPASS, AluInp.PREV_DELAY_4)   # z flop <- 0
    for k in range(6, 8):
        d[k].pass_through_alu()

    st = UopConfig()
    base_inputs(st)
    st.require_inp0 = ENABLE
    st.require_inp1 = ENABLE
    st.trigger = (Trigger.SRC_TENSOR_DONE, Trigger.NONE, Trigger.NONE)
    st.next_uop = (0, 0, 0)
    st.enable_output(OutSel.ALU_OUT, OutPath.WR0_LO)     # y_even
    st.enable_output(OutSel.DELAY_0, OutPath.WR0_HI)     # y_odd = z
    d = st.datapath_config
    d[0].enable_alu(AluOp.MULTIPLY, AluInp.PREV_DELAY_0, AluInp.PREV_DELAY_3)
    d[0].enable_delay_from_src(DelayInp.PREV_ALU_OUT, 0)  # capture x_e
    d[0].pass_through_delay(1, 2, 3)           # x_o, k_o, C
    d[1].enable_alu(AluOp.MULTIPLY, AluInp.PREV_DELAY_2, AluInp.PREV_DELAY_3)
    d[1].pass_through_delay(0, 1)              # x_e, x_o
    d[1].enable_delay_from_src(DelayInp.PREV_ALU_OUT, 3)  # capture k'_e
    d[2].enable_alu(AluOp.MULTIPLY, AluInp.PREV_DELAY_0, AluInp.PREV_DELAY_3)
    d[2].pass_through_delay(1)                 # x_o
    d[2].enable_delay_from_src(DelayInp.PREV_ALU_OUT, 2)  # capture k'_o
    d[3].enable_alu(AluOp.MULTIPLY, AluInp.PREV_DELAY_1, AluInp.PREV_DELAY_2)
    d[3].enable_delay_from_src(DelayInp.PREV_ALU_OUT, 3)  # capture m_e
    d[4].enable_alu(AluOp.ADD, AluInp.PREV_ALU_OUT, AluInp.PREV_DELAY_3)  # p
    d[4].enable_delay_from_src(DelayInp.PREV_ALU_OUT, 1)  # capture m_o
    d[5].enable_alu(AluOp.ADD, AluInp.CURR_ALU_OUT, AluInp.PREV_ALU_OUT)  # z
    d[5].pass_through_delay(1)                 # m_o
    d[6].enable_alu(AluOp.SUBTRACT, AluInp.PREV_ALU_OUT, AluInp.PREV_DELAY_1)
    d[6].enable_delay_from_src(DelayInp.PREV_ALU_OUT, 0)  # capture z
    d[7].pass_through_alu()
    d[7].pass_through_delay(0)
    return [seed, st]


def _mk_v2xs_uops():
    """vc = cumsum(k*C) pair-scan; SRC_1 is required and drained (keeps the
    op in the TwoSrc perf class) but ignored."""
    from concourse.dve_uop import (
        ENABLE, AluInp, AluOp, DelayInp, InpSel, OutPath, OutSel,
        Trigger, UopConfig,
    )

    def base_inputs(u):
        u.enable_input(InpSel.SRC_0, 0)        # k_e -> stage0 ALU A
        u.enable_input(InpSel.SRC_1, 1)        # drained, value ignored
        u.enable_input(InpSel.SRC_0_HI, 2)     # k_o -> PREV_DELAY_1
        u.enable_input(InpSel.CONST_0, 3)      # C   -> PREV_DELAY_2
        u.enable_input(InpSel.ZERO, 4)         # 0   -> PREV_DELAY_3

    seed = UopConfig()
    base_inputs(seed)
    seed.repeat_count = 1
    seed.trigger = (Trigger.COUNT, Trigger.NONE, Trigger.NONE)
    seed.next_uop = (1, 0, 0)
    d = seed.datapath_config
    for k in range(3):
        d[k].pass_through_alu()
        d[k].pass_through_delay(3)
    d[3].enable_alu(AluOp.BYPASS, AluInp.PREV_DELAY_3)   # w flop <- 0
    for k in range(4, 8):
        d[k].pass_through_alu()

    st = UopConfig()
    base_inputs(st)
    st.require_inp0 = ENABLE
    st.require_inp1 = ENABLE
    st.trigger = (Trigger.SRC_TENSOR_DONE, Trigger.NONE, Trigger.NONE)
    st.next_uop = (0, 0, 0)
    st.enable_output(OutSel.ALU_OUT, OutPath.WR0_LO)     # y_even
    st.enable_output(OutSel.DELAY_0, OutPath.WR0_HI)     # y_odd = w
    d = st.datapath_config
    d[0].enable_alu(AluOp.MULTIPLY, AluInp.PREV_ALU_OUT, AluInp.PREV_DELAY_2)
    d[0].pass_through_delay(1, 2)              # k_o, C
    d[1].enable_alu(AluOp.MULTIPLY, AluInp.PREV_DELAY_1, AluInp.PREV_DELAY_2)
    d[1].enable_delay_from_src(DelayInp.PREV_ALU_OUT, 0)  # capture k'_e
    d[2].enable_alu(AluOp.ADD, AluInp.PREV_ALU_OUT, AluInp.PREV_DELAY_0)  # q
    d[2].enable_delay_from_src(DelayInp.PREV_ALU_OUT, 1)  # capture k'_o
    d[3].enable_alu(AluOp.ADD, AluInp.CURR_ALU_OUT, AluInp.PREV_ALU_OUT)  # w
    d[3].pass_through_delay(1)                 # k'_o
    d[4].enable_alu(AluOp.SUBTRACT, AluInp.PREV_ALU_OUT, AluInp.PREV_DELAY_1)
    d[4].enable_delay_from_src(DelayInp.PREV_ALU_OUT, 0)  # capture w
    for k in range(5, 8):
        d[k].pass_through_alu()
        d[k].pass_through_delay(0)
    return [seed, st]


def _register_pair_ops():
    """Register the scaled pair-scan DveOps; the hand 2x program is injected
    via the compile cache so DveOp.compile returns it table-generation-time."""
    import concourse.dve_ops as dve_ops
    from concourse.dve_ops import DveOp, _COMPILE_CACHE
    from concourse.dve_spec import AluOp, C0, Spec, Src0, Src1, lower, scan
    from concourse.dve_uop import DveOpSpec

    def mk(name, spec, uops_2x):
        for o in dve_ops.OPS:
            if o.name == name:
                return o
        op = DveOp(name, spec, subdim=False, uops_sha={})
        dve_ops.OPS.append(op)
        dve_ops.CUSTOM_DVE_SPECS[name] = spec
        dve_ops._SUB_OPCODE_FOR_NAME[name] = (
            dve_ops._CUSTOM_DVE_ROW_BASE + len(dve_ops.OPS) - 1
        )
        assert dve_ops._SUB_OPCODE_FOR_NAME[name] < 0x20
        for ver in ("v3", "v4"):
            s = DveOpSpec(
                name=name,
                opcode=dve_ops.get_dve_sub_opcode(name),
                uops=lower(spec, ver=ver),
                uops_2x=uops_2x,
                perf_max=1,
                rd1_en=True,
            )
            op.uops_sha[ver] = s.sha(ver)
            _COMPILE_CACHE[(name, ver)] = s
        return op

    pv = mk(
        "PV2XS_ANT",
        Spec(
            body=scan(AluOp.ADD, Src0 * (Src1 * C0)),
            reference=lambda in0, in1, s0, s1, imm2: np.cumsum(
                in0.astype(np.float32) * (in1.astype(np.float32) * np.float32(s0)),
                axis=-1, dtype=np.float32,
            ),
        ),
        _mk_pv2xs_uops(),
    )
    v = mk(
        "V2XS_ANT",
        Spec(
            body=scan(AluOp.ADD, Src0 * C0),
            reference=lambda in0, in1, s0, s1, imm2: np.cumsum(
                in0.astype(np.float32) * np.float32(s0), axis=-1,
                dtype=np.float32,
            ),
        ),
        _mk_v2xs_uops(),
    )
    return pv, v


def _build(rows=ROWS, time=TIME, bufs=3, reps=1):
    import concourse.tile as tile
    import concourse.mybir as mybir

    pv_op, v_op = _register_pair_ops()
    nc = _single_act_set_bacc()("TRN2", target_bir_lowering=False, debug=False)
    f32 = mybir.dt.float32
    f16 = mybir.dt.float16
    u8 = mybir.dt.uint8
    x8d = nc.dram_tensor("stock_codes", [rows, time], u8, kind="ExternalInput").ap()
    k8d = nc.dram_tensor("volume_codes", [rows, time], u8, kind="ExternalInput").ap()
    out = nc.dram_tensor("vwap_f16", [rows, time], f16, kind="ExternalOutput").ap()

    Ln = mybir.ActivationFunctionType.Ln
    Exp = mybir.ActivationFunctionType.Exp

    n_tiles = rows // P
    total = n_tiles * reps
    LOOKAHEAD = 2
    with tile.TileContext(nc) as tc:
        with tc.tile_pool(name="big", bufs=bufs) as big:
            pend = {}

            def issue_loads(j):
                r0 = (j % n_tiles) * P
                xt = big.tile([P, time], f16, tag="xt")
                kt = big.tile([P, time], f16, tag="kt")
                nc.gpsimd.dma_start(xt[:], x8d[r0 : r0 + P, :])
                nc.gpsimd.dma_start(kt[:], k8d[r0 : r0 + P, :])
                pend[j] = (xt, kt)

            for j in range(min(LOOKAHEAD, total)):
                issue_loads(j)
            for i in range(total):
                if i + LOOKAHEAD < total:
                    issue_loads(i + LOOKAHEAD)
                xt, kt = pend.pop(i)
                r0 = (i % n_tiles) * P
                pv = big.tile([P, time], f16, tag="pv")
                vc = big.tile([P, time], f16, tag="vc")
                ln = big.tile([P, time], f32, tag="ln")
                oc = big.tile([P, time], f16, tag="oc")
                i1 = nc.vector._custom_dve(
                    pv_op, out=pv[:], in0=xt[:], in1=kt[:], s0=C_PV
                )
                i2 = nc.vector._custom_dve(
                    v_op, out=vc[:], in0=kt[:], in1=xt[:], s0=C_V
                )
                i1.ins.perf_max = 1
                i2.ins.perf_max = 1
                nc.scalar.activation(ln[:], vc[:], Ln, scale=C_LN)
                nc.scalar.activation(vc[:], ln[:], Exp, scale=-1.0)
                nc.vector.tensor_mul(oc[:], pv[:], vc[:])
                nc.sync.dma_start(out[r0 : r0 + P, :], oc[:])
    nc.compile()
    return nc


def _get_nc():
    if "nc" not in _CACHE:
        _CACHE["nc"] = _build()
    return _CACHE["nc"]


def _prep_inputs(stock_paths, volume_paths):
    x8 = np.clip(
        np.round((stock_paths - np.float32(30.0)) * np.float32(255.0 / 30.0)),
        0, 255,
    ).astype(np.uint8)
    k8 = np.clip(
        np.round(volume_paths * np.float32(255.0 / 8e6)), 0, 255
    ).astype(np.uint8)
    return x8, k8


def kernel(stock_paths: np.ndarray, volume_paths: np.ndarray) -> np.ndarray:
    from concourse.bass_utils import run_bass_kernel_spmd

    stock_paths = np.ascontiguousarray(stock_paths, dtype=np.float32)
    volume_paths = np.ascontiguousarray(volume_paths, dtype=np.float32)
    assert stock_paths.shape == (NUM_PATHS, TIME)

    x8, k8 = _prep_inputs(stock_paths, volume_paths)
    nc = _get_nc()
    in_maps = [
        {
            "stock_codes": x8[i * ROWS : (i + 1) * ROWS],
            "volume_codes": k8[i * ROWS : (i + 1) * ROWS],
        }
        for i in range(N_CORES)
    ]
    res = run_bass_kernel_spmd(nc, in_maps, core_ids=list(range(N_CORES)))
    return np.concatenate(
        [
            r["vwap_f16"].astype(np.float32) * np.float32(30.0 / 255.0)
            + np.float32(30.0)
            for r in res.results
        ],
        axis=0,
    )
